# revision 21
# baseline (speedup 1.0000x reference)
"""Class-balanced focal segmentation loss on 8 trn2 NeuronCores.

Input: pred/target [8, 21, 512, 512] f32. Output: f32 scalar.

Sharding: data-parallel over B — core i handles batch image i.
Per-core layout: the [21, 262144] plane is processed in tiles of
[126, 2048] (6 pixel-groups x 21 classes stacked on partitions).
Per 128-pixel chunk, channel reductions (Z = sum_c exp(x), s = sum_c
t*x) are computed with the data chunk as the matmul *stationary*
operand and a block-ones [126, 6] matrix as the moving operand, so the
per-pixel partials are born pixel-major [128, 6] in PSUM.  Per-pixel
focal math runs on those small pixel-major tiles.  Per-class masked
sums and counts come from one accumulating matmul per chunk:
[rv|0..|1][128,7]^T @ T^T[128,126] += [7,126].  The host folds the 6
groups and 8 cores and applies the class-balanced weighting.
"""

import sys

for _p in ("/opt/trn_rl_repo",):
    if _p not in sys.path:
        sys.path.insert(0, _p)

from contextlib import ExitStack

import ml_dtypes
import numpy as np

import concourse.bacc as bacc
import concourse.bass as bass
import concourse.tile as tile
from concourse import mybir

B, C, H, W = 8, 21, 512, 512
HW = H * W  # 262144 pixels per core
GAMMA = 2.0

F = 2048  # pixels per group-column block
KW = 128  # chunk width (pixels per matmul chunk)
KCH = F // KW  # 16 chunks per tile
G_FULL = 6  # pixel groups per full tile (6*21 = 126 partitions)
TILE_PX = G_FULL * F  # 12288
N_FULL = HW // TILE_PX  # 21 full tiles
G_REM = (HW - N_FULL * TILE_PX) // F  # 2 remainder groups
BATCH = 5  # tiles per pixel-math batch

f32 = mybir.dt.float32
bf16 = mybir.dt.bfloat16
AF = mybir.ActivationFunctionType


def _patch_act_tables():
    import concourse.hw_specs as hw_specs

    if getattr(hw_specs, "_act_tables_patched", False):
        return
    orig = hw_specs.get_activation_tables

    def patched(module_arch):
        tables = orig(module_arch)
        for name, funcs in tables.items():
            if name != "natural_log_exp_and_others":
                funcs.discard(AF.Exp)
                funcs.discard(AF.Ln)
        return tables

    hw_specs.get_activation_tables = patched
    bacc.get_activation_tables = patched  # bacc imported the name directly

    hw_specs._act_tables_patched = True


def build_nc(hw_px=HW):
    """Build the per-core Bass kernel. hw_px can be shrunk for simulation."""
    _patch_act_tables()
    n_full = hw_px // TILE_PX
    g_rem = (hw_px - n_full * TILE_PX) // F
    assert n_full * TILE_PX + g_rem * F == hw_px

    nc = bacc.Bacc()
    pred = nc.dram_tensor("pred", [C, hw_px], f32, kind="ExternalInput")
    targ = nc.dram_tensor("target", [C, hw_px], f32, kind="ExternalInput")
    s1_d = nc.dram_tensor("s1", [G_FULL * C, G_FULL], bf16, kind="ExternalInput")
    id_d = nc.dram_tensor("ident", [128, 128], bf16, kind="ExternalInput")
    out_d = nc.dram_tensor("out", [7, G_FULL * C], f32, kind="ExternalOutput")

    nchunks = hw_px // F
    pred_r = pred.rearrange("c (n e) -> n c e", e=F)  # [nchunks, 21, F]
    targ_r = targ.rearrange("c (n e) -> n c e", e=F)

    # (start chunk, group count) per tile
    tiles = [(t * G_FULL, G_FULL) for t in range(n_full)]
    if g_rem:
        tiles.append((n_full * G_FULL, g_rem))

    n_cls_mm = len(tiles) * KCH  # total class-matmuls

    with tile.TileContext(nc) as tc, ExitStack() as ctx:
        const = ctx.enter_context(tc.tile_pool(name="const", bufs=1))
        iop = ctx.enter_context(tc.tile_pool(name="io", bufs=4))
        midp = ctx.enter_context(tc.tile_pool(name="mid", bufs=3))
        ttp = ctx.enter_context(tc.tile_pool(name="ttsb", bufs=4))
        pxp = ctx.enter_context(tc.tile_pool(name="px", bufs=4))
        psA = ctx.enter_context(tc.tile_pool(name="psA", bufs=2, space="PSUM"))
        psT = ctx.enter_context(tc.tile_pool(name="psT", bufs=2, space="PSUM"))
        psC = ctx.enter_context(tc.tile_pool(name="psC", bufs=1, space="PSUM"))

        s1_sb = const.tile([G_FULL * C, G_FULL], bf16)
        nc.sync.dma_start(s1_sb[:], s1_d[:])
        id_sb = const.tile([128, 128], bf16)
        nc.sync.dma_start(id_sb[:], id_d[:])
        # junk moving operand + junk PSUM sink for the HAM heater matmuls
        junk_sb = const.tile([128, 512], bf16)
        nc.vector.memset(junk_sb[:], 0.0)
        junk_ps = psT.tile([128, 512], f32, tag="junk", bufs=1)

        cls_ps = psC.tile([7, G_FULL * C], f32)
        mm_idx = 0  # running index over class-matmuls for start/stop flags

        for ti, (c0, g) in enumerate(tiles):
            p = C * g
            x = iop.tile([p, F], f32, tag="x")
            nc.sync.dma_start(x[:], pred_r[c0 : c0 + g])
            t = iop.tile([p, F], f32, tag="t")
            nc.gpsimd.dma_start(t[:], targ_r[c0 : c0 + g])
            e = midp.tile([p, F], bf16, tag="e")
            nc.scalar.activation(e[:], x[:], AF.Exp)
            tx = midp.tile([p, F], bf16, tag="tx")
            nc.vector.tensor_mul(tx[:], t[:], x[:])
            tb = midp.tile([p, F], bf16, tag="tb")
            nc.vector.tensor_copy(tb[:], t[:])

            zs_ps = psA.tile([128, 2, KCH, g], f32, tag="zs")
            tt_ps = psT.tile([128, KCH * 128], bf16, tag="ttps")
            for k in range(KCH):
                ch = slice(k * KW, (k + 1) * KW)
                nc.tensor.matmul(
                    zs_ps[:, 0, k, :], e[:, ch], s1_sb[0:p, 0:g],
                    start=True, stop=True,
                )
                nc.tensor.matmul(
                    zs_ps[:, 1, k, :], tx[:, ch], s1_sb[0:p, 0:g],
                    start=True, stop=True,
                )
                nc.tensor.transpose(
                    tt_ps[:, k * KW : k * KW + p], tb[:, ch], id_sb[0:p, 0:p]
                )
                if k in (5, 11):
                    # HAM heater: dense 512-col matmul keeps the PE array
                    # busy enough to hold the 2.4 GHz clock state
                    nc.tensor.matmul(
                        junk_ps[0:128, :], e[:, ch], junk_sb[0:p, :],
                        start=True, stop=True, skip_group_check=True,
                    )
            tt_sb = ttp.tile([128, KCH, C * G_FULL], bf16, tag="ttsb")
            tt_src = tt_ps[:].rearrange("q (k w) -> q k w", w=KW)[:, :, 0:p]
            nc.scalar.copy(tt_sb[:, :, 0:p], tt_src)

            # pixel-major focal math straight off PSUM
            lnz = pxp.tile([128, KCH, g], f32, tag="lnz")
            nc.scalar.activation(lnz[:], zs_ps[:, 0], AF.Ln)
            lp = pxp.tile([128, KCH, g], f32, tag="lp")
            nc.vector.tensor_sub(lp[:], zs_ps[:, 1], lnz[:])
            pp = pxp.tile([128, KCH, g], f32, tag="pp")
            nc.scalar.activation(pp[:], lp[:], AF.Exp)
            # rv = (1-p)^2*lp = (p-1)*((p-1)*lp) in two fused stt ops
            u = pxp.tile([128, KCH, g], f32, tag="u")
            nc.vector.scalar_tensor_tensor(
                u[:], pp[:], -1.0, lp[:],
                op0=mybir.AluOpType.add, op1=mybir.AluOpType.mult,
            )
            # rv into the [rv | pad | ones] stationary. Whole-tile memset to
            # 1.0: col 6 becomes the n_c ones column; cols g..6 of the
            # remainder tile hit accumulator cells the host never reads.
            rv7 = pxp.tile([128, KCH, 7], bf16, tag="rv7")
            nc.vector.memset(rv7[:], 1.0)
            nc.vector.scalar_tensor_tensor(
                rv7[:, :, 0:g], pp[:], -1.0, u[:],
                op0=mybir.AluOpType.add, op1=mybir.AluOpType.mult,
            )

            for k in range(KCH):
                nc.tensor.matmul(
                    cls_ps[0:7, 0 : C * g],
                    rv7[:, k, :],
                    tt_sb[:, k, 0:p],
                    start=(mm_idx == 0),
                    stop=(mm_idx == n_cls_mm - 1),
                    skip_group_check=True,
                )
                mm_idx += 1

        cls_sb = const.tile([7, G_FULL * C], f32)
        nc.vector.tensor_copy(cls_sb[:], cls_ps[:])
        nc.sync.dma_start(out_d[:], cls_sb[:])

    return nc


def make_consts():
    s1 = np.zeros((G_FULL * C, G_FULL), dtype=ml_dtypes.bfloat16)
    for g in range(G_FULL):
        s1[g * C : (g + 1) * C, g] = 1
    ident = np.eye(128, dtype=ml_dtypes.bfloat16)
    return s1, ident


def finalize(cls_sum, hw_px=HW, n_cores=B):
    """Host-side fold of the per-core partials into the loss."""
    if cls_sum.shape[0] == 128:
        cls_sum = sum(cls_sum[32 * j : 32 * j + 7] for j in range(4))
    class_sum = np.zeros(C, dtype=np.float64)
    n_c = np.zeros(C, dtype=np.float64)
    for g in range(G_FULL):
        class_sum += cls_sum[g, g * C : (g + 1) * C]
        n_c += cls_sum[6, g * C : (g + 1) * C]
    n = float(n_cores) * float(hw_px)
    beta = np.float64(np.float32((n - 1.0) / n))
    w = (1.0 - beta) / (1.0 - np.power(beta, n_c) + 1e-6)
    return np.float32(-(w * class_sum).sum() / n)


_NC_CACHE = {}


def _get_nc():
    if "nc" not in _NC_CACHE:
        nc = build_nc()
        nc.finalize()
        _NC_CACHE["nc"] = nc
    return _NC_CACHE["nc"]


LAST_PROFILE = {}


def _install_trace_shims():
    """Provide the NTFF profile hook this image's antenv lacks, and keep
    artifact upload local."""
    import sys as _s
    import types

    try:
        import antenv.axon_hooks  # noqa: F401
    except ImportError:
        if "/root/.axon_site" not in _s.path:
            _s.path.insert(0, "/root/.axon_site")
        from trn_agent_boot.trn_boot import _ntff_profile_via_ctypes

        hook = _ntff_profile_via_ctypes("/opt/axon/libaxon_pjrt.so")
        mod = types.ModuleType("antenv.axon_hooks")
        mod.get_axon_ntff_profile_hook = lambda: hook
        mod.set_axon_ntff_profile_hook = lambda h: None
        _s.modules["antenv.axon_hooks"] = mod
    import concourse.bass_utils as bu

    bu.upload_artifacts = lambda tmpdir: str(tmpdir)


def kernel(pred, target, _trace=False):
    pred = np.ascontiguousarray(np.asarray(pred, dtype=np.float32)).reshape(B, C, HW)
    target = np.ascontiguousarray(np.asarray(target, dtype=np.float32)).reshape(
        B, C, HW
    )
    from concourse.bass_utils import run_bass_kernel_spmd

    nc = _get_nc()
    s1, ident = make_consts()
    in_maps = [
        {"pred": pred[i], "target": target[i], "s1": s1, "ident": ident}
        for i in range(B)
    ]
    kw = {}
    if _trace:
        import os

        import shutil

        _install_trace_shims()
        kw = {"tmpdir": "/root/problem/trace_out"}
        shutil.rmtree(kw["tmpdir"], ignore_errors=True)
        os.makedirs(kw["tmpdir"], exist_ok=True)
    rr = run_bass_kernel_spmd(nc, in_maps, list(range(B)), trace=_trace, **kw)
    LAST_PROFILE["exec_time_ns"] = rr.exec_time_ns
    LAST_PROFILE["trace"] = rr.instructions_and_trace
    LAST_PROFILE["profile_json"] = rr.profile_json
    cls = np.zeros((7, G_FULL * C), dtype=np.float64)
    for r in rr.results:
        cls += r["out"].astype(np.float64)
    return finalize(cls)


# revision 23
# speedup vs baseline: 1.0081x; 1.0081x over previous
"""Class-balanced focal segmentation loss on 8 trn2 NeuronCores.

Input: pred/target [8, 21, 512, 512] f32. Output: f32 scalar.

Sharding: data-parallel over B — core i handles batch image i.
Per-core layout: the [21, 262144] plane is processed in tiles of
[126, 2048] (6 pixel-groups x 21 classes stacked on partitions).
Per 128-pixel chunk, channel reductions (Z = sum_c exp(x), s = sum_c
t*x) are computed with the data chunk as the matmul *stationary*
operand and a block-ones [126, 6] matrix as the moving operand, so the
per-pixel partials are born pixel-major [128, 6] in PSUM.  Per-pixel
focal math runs on those small pixel-major tiles.  Per-class masked
sums and counts come from one accumulating matmul per chunk:
[rv|0..|1][128,7]^T @ T^T[128,126] += [7,126].  The host folds the 6
groups and 8 cores and applies the class-balanced weighting.
"""

import sys

for _p in ("/opt/trn_rl_repo",):
    if _p not in sys.path:
        sys.path.insert(0, _p)

from contextlib import ExitStack

import ml_dtypes
import numpy as np

import concourse.bacc as bacc
import concourse.bass as bass
import concourse.tile as tile
from concourse import mybir

B, C, H, W = 8, 21, 512, 512
HW = H * W  # 262144 pixels per core
GAMMA = 2.0

F = 2048  # pixels per group-column block
KW = 128  # chunk width (pixels per matmul chunk)
KCH = F // KW  # 16 chunks per tile
G_FULL = 6  # pixel groups per full tile (6*21 = 126 partitions)
TILE_PX = G_FULL * F  # 12288
N_FULL = HW // TILE_PX  # 21 full tiles
G_REM = (HW - N_FULL * TILE_PX) // F  # 2 remainder groups
BATCH = 5  # tiles per pixel-math batch

f32 = mybir.dt.float32
bf16 = mybir.dt.bfloat16
AF = mybir.ActivationFunctionType


def _patch_act_tables():
    import concourse.hw_specs as hw_specs

    if getattr(hw_specs, "_act_tables_patched", False):
        return
    orig = hw_specs.get_activation_tables

    def patched(module_arch):
        tables = orig(module_arch)
        for name, funcs in tables.items():
            if name != "natural_log_exp_and_others":
                funcs.discard(AF.Exp)
                funcs.discard(AF.Ln)
        return tables

    hw_specs.get_activation_tables = patched
    bacc.get_activation_tables = patched  # bacc imported the name directly

    hw_specs._act_tables_patched = True


def build_nc(hw_px=HW):
    """Build the per-core Bass kernel. hw_px can be shrunk for simulation."""
    _patch_act_tables()
    n_full = hw_px // TILE_PX
    g_rem = (hw_px - n_full * TILE_PX) // F
    assert n_full * TILE_PX + g_rem * F == hw_px

    nc = bacc.Bacc()
    pred = nc.dram_tensor("pred", [C, hw_px], f32, kind="ExternalInput")
    targ = nc.dram_tensor("target", [C, hw_px], f32, kind="ExternalInput")
    s1_d = nc.dram_tensor("s1", [G_FULL * C, G_FULL], bf16, kind="ExternalInput")
    id_d = nc.dram_tensor("ident", [128, 128], bf16, kind="ExternalInput")
    out_d = nc.dram_tensor("out", [7, G_FULL * C], f32, kind="ExternalOutput")

    nchunks = hw_px // F
    pred_r = pred.rearrange("c (n e) -> n c e", e=F)  # [nchunks, 21, F]
    targ_r = targ.rearrange("c (n e) -> n c e", e=F)

    # (start chunk, group count) per tile
    tiles = [(t * G_FULL, G_FULL) for t in range(n_full)]
    if g_rem:
        tiles.append((n_full * G_FULL, g_rem))

    n_cls_mm = len(tiles) * KCH  # total class-matmuls

    with tile.TileContext(nc) as tc, ExitStack() as ctx:
        const = ctx.enter_context(tc.tile_pool(name="const", bufs=1))
        iop = ctx.enter_context(tc.tile_pool(name="io", bufs=4))
        midp = ctx.enter_context(tc.tile_pool(name="mid", bufs=3))
        ttp = ctx.enter_context(tc.tile_pool(name="ttsb", bufs=4))
        pxp = ctx.enter_context(tc.tile_pool(name="px", bufs=4))
        psA = ctx.enter_context(tc.tile_pool(name="psA", bufs=3, space="PSUM"))
        psT = ctx.enter_context(tc.tile_pool(name="psT", bufs=2, space="PSUM"))
        psC = ctx.enter_context(tc.tile_pool(name="psC", bufs=1, space="PSUM"))

        s1_sb = const.tile([G_FULL * C, G_FULL], bf16)
        nc.sync.dma_start(s1_sb[:], s1_d[:])
        id_sb = const.tile([128, 128], bf16)
        nc.sync.dma_start(id_sb[:], id_d[:])

        cls_ps = psC.tile([7, G_FULL * C], f32)
        mm_idx = 0  # running index over class-matmuls for start/stop flags

        for ti, (c0, g) in enumerate(tiles):
            p = C * g
            x = iop.tile([p, F], f32, tag="x")
            nc.sync.dma_start(x[:], pred_r[c0 : c0 + g])
            t = iop.tile([p, F], f32, tag="t")
            nc.gpsimd.dma_start(t[:], targ_r[c0 : c0 + g])
            e = midp.tile([p, F], bf16, tag="e")
            nc.scalar.activation(e[:], x[:], AF.Exp)
            tb = midp.tile([p, F], bf16, tag="tb")
            nc.vector.tensor_copy(tb[:], t[:])
            tx = midp.tile([p, F], bf16, tag="tx")
            nc.vector.tensor_mul(tx[:], t[:], x[:])

            zs_ps = psA.tile([128, 2, KCH, g], f32, tag="zs")
            tt_ps = psT.tile([128, KCH * 128], bf16, tag="ttps")
            for k in range(KCH):
                ch = slice(k * KW, (k + 1) * KW)
                nc.tensor.matmul(
                    zs_ps[:, 0, k, :], e[:, ch], s1_sb[0:p, 0:g],
                    start=True, stop=True,
                )
                nc.tensor.matmul(
                    zs_ps[:, 1, k, :], tx[:, ch], s1_sb[0:p, 0:g],
                    start=True, stop=True,
                )
                nc.tensor.transpose(
                    tt_ps[:, k * KW : k * KW + p], tb[:, ch], id_sb[0:p, 0:p]
                )
            tt_sb = ttp.tile([128, KCH, C * G_FULL], bf16, tag="ttsb")
            tt_src = tt_ps[:].rearrange("q (k w) -> q k w", w=KW)[:, :, 0:p]
            nc.scalar.copy(tt_sb[:, :, 0:p], tt_src)

            # pixel-major focal math straight off PSUM
            lnz = pxp.tile([128, KCH, g], f32, tag="lnz")
            nc.scalar.activation(lnz[:], zs_ps[:, 0], AF.Ln)
            lp = pxp.tile([128, KCH, g], f32, tag="lp")
            nc.vector.tensor_sub(lp[:], zs_ps[:, 1], lnz[:])
            pp = pxp.tile([128, KCH, g], f32, tag="pp")
            nc.scalar.activation(pp[:], lp[:], AF.Exp)
            # rv = (1-p)^2*lp = (p-1)*((p-1)*lp) in two fused stt ops
            u = pxp.tile([128, KCH, g], f32, tag="u")
            nc.vector.scalar_tensor_tensor(
                u[:], pp[:], -1.0, lp[:],
                op0=mybir.AluOpType.add, op1=mybir.AluOpType.mult,
            )
            # rv into the [rv | pad | ones] stationary. Whole-tile memset to
            # 1.0: col 6 becomes the n_c ones column; cols g..6 of the
            # remainder tile hit accumulator cells the host never reads.
            rv7 = pxp.tile([128, KCH, 7], bf16, tag="rv7")
            nc.vector.memset(rv7[:], 1.0)
            nc.vector.scalar_tensor_tensor(
                rv7[:, :, 0:g], pp[:], -1.0, u[:],
                op0=mybir.AluOpType.add, op1=mybir.AluOpType.mult,
            )

            for k in range(KCH):
                nc.tensor.matmul(
                    cls_ps[0:7, 0 : C * g],
                    rv7[:, k, :],
                    tt_sb[:, k, 0:p],
                    start=(mm_idx == 0),
                    stop=(mm_idx == n_cls_mm - 1),
                    skip_group_check=True,
                )
                mm_idx += 1

        cls_sb = const.tile([7, G_FULL * C], f32)
        nc.vector.tensor_copy(cls_sb[:], cls_ps[:])
        nc.sync.dma_start(out_d[:], cls_sb[:])

    return nc


def make_consts():
    s1 = np.zeros((G_FULL * C, G_FULL), dtype=ml_dtypes.bfloat16)
    for g in range(G_FULL):
        s1[g * C : (g + 1) * C, g] = 1
    ident = np.eye(128, dtype=ml_dtypes.bfloat16)
    return s1, ident


def finalize(cls_sum, hw_px=HW, n_cores=B):
    """Host-side fold of the per-core partials into the loss."""
    if cls_sum.shape[0] == 128:
        cls_sum = sum(cls_sum[32 * j : 32 * j + 7] for j in range(4))
    class_sum = np.zeros(C, dtype=np.float64)
    n_c = np.zeros(C, dtype=np.float64)
    for g in range(G_FULL):
        class_sum += cls_sum[g, g * C : (g + 1) * C]
        n_c += cls_sum[6, g * C : (g + 1) * C]
    n = float(n_cores) * float(hw_px)
    beta = np.float64(np.float32((n - 1.0) / n))
    w = (1.0 - beta) / (1.0 - np.power(beta, n_c) + 1e-6)
    return np.float32(-(w * class_sum).sum() / n)


_NC_CACHE = {}


def _get_nc():
    if "nc" not in _NC_CACHE:
        nc = build_nc()
        nc.finalize()
        _NC_CACHE["nc"] = nc
    return _NC_CACHE["nc"]


LAST_PROFILE = {}


def _install_trace_shims():
    """Provide the NTFF profile hook this image's antenv lacks, and keep
    artifact upload local."""
    import sys as _s
    import types

    try:
        import antenv.axon_hooks  # noqa: F401
    except ImportError:
        if "/root/.axon_site" not in _s.path:
            _s.path.insert(0, "/root/.axon_site")
        from trn_agent_boot.trn_boot import _ntff_profile_via_ctypes

        hook = _ntff_profile_via_ctypes("/opt/axon/libaxon_pjrt.so")
        mod = types.ModuleType("antenv.axon_hooks")
        mod.get_axon_ntff_profile_hook = lambda: hook
        mod.set_axon_ntff_profile_hook = lambda h: None
        _s.modules["antenv.axon_hooks"] = mod
    import concourse.bass_utils as bu

    bu.upload_artifacts = lambda tmpdir: str(tmpdir)


def kernel(pred, target, _trace=False):
    pred = np.ascontiguousarray(np.asarray(pred, dtype=np.float32)).reshape(B, C, HW)
    target = np.ascontiguousarray(np.asarray(target, dtype=np.float32)).reshape(
        B, C, HW
    )
    from concourse.bass_utils import run_bass_kernel_spmd

    nc = _get_nc()
    s1, ident = make_consts()
    in_maps = [
        {"pred": pred[i], "target": target[i], "s1": s1, "ident": ident}
        for i in range(B)
    ]
    kw = {}
    if _trace:
        import os

        import shutil

        _install_trace_shims()
        kw = {"tmpdir": "/root/problem/trace_out"}
        shutil.rmtree(kw["tmpdir"], ignore_errors=True)
        os.makedirs(kw["tmpdir"], exist_ok=True)
    rr = run_bass_kernel_spmd(nc, in_maps, list(range(B)), trace=_trace, **kw)
    LAST_PROFILE["exec_time_ns"] = rr.exec_time_ns
    LAST_PROFILE["trace"] = rr.instructions_and_trace
    LAST_PROFILE["profile_json"] = rr.profile_json
    cls = np.zeros((7, G_FULL * C), dtype=np.float64)
    for r in rr.results:
        cls += r["out"].astype(np.float64)
    return finalize(cls)


# revision 26
# speedup vs baseline: 1.0515x; 1.0430x over previous
"""Class-balanced focal segmentation loss on 8 trn2 NeuronCores.

Input: pred/target [8, 21, 512, 512] f32. Output: f32 scalar.

Sharding: data-parallel over B — core i handles batch image i.
Per-core layout: the [21, 262144] plane is processed in tiles of
[126, 2048] (6 pixel-groups x 21 classes stacked on partitions).
Per 128-pixel chunk, channel reductions (Z = sum_c exp(x), s = sum_c
t*x) are computed with the data chunk as the matmul *stationary*
operand and a block-ones [126, 6] matrix as the moving operand, so the
per-pixel partials are born pixel-major [128, 6] in PSUM.  Per-pixel
focal math runs on those small pixel-major tiles.  Per-class masked
sums and counts come from one accumulating matmul per chunk:
[rv|0..|1][128,7]^T @ T^T[128,126] += [7,126].  The host folds the 6
groups and 8 cores and applies the class-balanced weighting.
"""

import sys

for _p in ("/opt/trn_rl_repo",):
    if _p not in sys.path:
        sys.path.insert(0, _p)

from contextlib import ExitStack

import ml_dtypes
import numpy as np

import concourse.bacc as bacc
import concourse.bass as bass
import concourse.tile as tile
from concourse import mybir

B, C, H, W = 8, 21, 512, 512
HW = H * W  # 262144 pixels per core
GAMMA = 2.0

F = 2048  # pixels per group-column block
KW = 128  # chunk width (pixels per matmul chunk)
KCH = F // KW  # 16 chunks per tile
G_FULL = 6  # pixel groups per full tile (6*21 = 126 partitions)
TILE_PX = G_FULL * F  # 12288
N_FULL = HW // TILE_PX  # 21 full tiles
G_REM = (HW - N_FULL * TILE_PX) // F  # 2 remainder groups
BATCH = 5  # tiles per pixel-math batch

f32 = mybir.dt.float32
bf16 = mybir.dt.bfloat16
AF = mybir.ActivationFunctionType


def _patch_act_tables():
    import concourse.hw_specs as hw_specs

    if getattr(hw_specs, "_act_tables_patched", False):
        return
    orig = hw_specs.get_activation_tables

    def patched(module_arch):
        tables = orig(module_arch)
        for name, funcs in tables.items():
            if name != "natural_log_exp_and_others":
                funcs.discard(AF.Exp)
                funcs.discard(AF.Ln)
        return tables

    hw_specs.get_activation_tables = patched
    bacc.get_activation_tables = patched  # bacc imported the name directly

    hw_specs._act_tables_patched = True


def build_nc(hw_px=HW):
    """Build the per-core Bass kernel. hw_px can be shrunk for simulation."""
    _patch_act_tables()
    n_full = hw_px // TILE_PX
    g_rem = (hw_px - n_full * TILE_PX) // F
    assert n_full * TILE_PX + g_rem * F == hw_px

    nc = bacc.Bacc()
    pred = nc.dram_tensor("pred", [C, hw_px], f32, kind="ExternalInput")
    targ = nc.dram_tensor("target", [C, hw_px], f32, kind="ExternalInput")
    s1_d = nc.dram_tensor("s1", [G_FULL * C, G_FULL], bf16, kind="ExternalInput")
    id_d = nc.dram_tensor("ident", [128, 128], bf16, kind="ExternalInput")
    out_d = nc.dram_tensor("out", [7, G_FULL * C], f32, kind="ExternalOutput")

    nchunks = hw_px // F
    pred_r = pred.rearrange("c (n e) -> n c e", e=F)  # [nchunks, 21, F]
    targ_r = targ.rearrange("c (n e) -> n c e", e=F)

    # (start chunk, group count) per tile
    tiles = [(t * G_FULL, G_FULL) for t in range(n_full)]
    if g_rem:
        tiles.append((n_full * G_FULL, g_rem))

    n_cls_mm = len(tiles) * KCH  # total class-matmuls

    with tile.TileContext(nc) as tc, ExitStack() as ctx:
        const = ctx.enter_context(tc.tile_pool(name="const", bufs=1))
        iop = ctx.enter_context(tc.tile_pool(name="io", bufs=4))
        midp = ctx.enter_context(tc.tile_pool(name="mid", bufs=3))
        ttp = ctx.enter_context(tc.tile_pool(name="ttsb", bufs=4))
        pxp = ctx.enter_context(tc.tile_pool(name="px", bufs=4))
        psA = ctx.enter_context(tc.tile_pool(name="psA", bufs=3, space="PSUM"))
        psT = ctx.enter_context(tc.tile_pool(name="psT", bufs=2, space="PSUM"))
        psC = ctx.enter_context(tc.tile_pool(name="psC", bufs=1, space="PSUM"))

        s1_sb = const.tile([G_FULL * C, G_FULL], bf16)
        nc.sync.dma_start(s1_sb[:], s1_d[:])
        id_sb = const.tile([128, 128], bf16)
        nc.sync.dma_start(id_sb[:], id_d[:])

        cls_ps = psC.tile([7, G_FULL * C], f32)
        mm_idx = 0  # running index over class-matmuls for start/stop flags

        for ti, (c0, g) in enumerate(tiles):
            p = C * g
            x = iop.tile([p, F], f32, tag="x")
            nc.sync.dma_start(x[:], pred_r[c0 : c0 + g])
            t = iop.tile([p, F], f32, tag="t")
            nc.gpsimd.dma_start(t[:], targ_r[c0 : c0 + g])
            e = midp.tile([p, F], bf16, tag="e")
            nc.scalar.activation(e[:], x[:], AF.Exp)
            tb = midp.tile([p, F], bf16, tag="tb")
            nc.vector.tensor_copy(tb[:], t[:])
            tx = midp.tile([p, F], bf16, tag="tx")
            nc.vector.tensor_mul(tx[:], t[:], x[:])

            zs_ps = psA.tile([128, 2, KCH, g], f32, tag="zs")
            tt_ps = psT.tile([128, KCH * 128], bf16, tag="ttps")
            for k in range(KCH):
                ch = slice(k * KW, (k + 1) * KW)
                nc.tensor.matmul(
                    zs_ps[:, 0, k, :], e[:, ch], s1_sb[0:p, 0:g],
                    start=True, stop=True,
                )
                nc.tensor.matmul(
                    zs_ps[:, 1, k, :], tx[:, ch], s1_sb[0:p, 0:g],
                    start=True, stop=True,
                )
                nc.tensor.transpose(
                    tt_ps[:, k * KW : k * KW + p], tb[:, ch], id_sb[0:p, 0:p]
                )
            tt_sb = ttp.tile([128, KCH, C * G_FULL], bf16, tag="ttsb")
            tt_src = tt_ps[:].rearrange("q (k w) -> q k w", w=KW)[:, :, 0:p]
            nc.scalar.copy(tt_sb[:, :, 0:p], tt_src)

            # pixel-major focal math straight off PSUM
            lnz = pxp.tile([128, KCH, g], f32, tag="lnz")
            nc.scalar.activation(lnz[:], zs_ps[:, 0], AF.Ln)
            lp = pxp.tile([128, KCH, g], f32, tag="lp")
            nc.vector.tensor_sub(lp[:], zs_ps[:, 1], lnz[:])
            pp = pxp.tile([128, KCH, g], f32, tag="pp")
            nc.scalar.activation(pp[:], lp[:], AF.Exp)
            # rv = (1-p)^2*lp = (p-1)*((p-1)*lp) in two fused stt ops
            u = pxp.tile([128, KCH, g], f32, tag="u")
            nc.vector.scalar_tensor_tensor(
                u[:], pp[:], -1.0, lp[:],
                op0=mybir.AluOpType.add, op1=mybir.AluOpType.mult,
            )
            # rv into the [rv | pad | ones] stationary. Whole-tile memset to
            # 1.0: col 6 becomes the n_c ones column; cols g..6 of the
            # remainder tile hit accumulator cells the host never reads.
            rv7 = pxp.tile([128, KCH, 7], bf16, tag="rv7")
            nc.vector.memset(rv7[:], 1.0)
            nc.vector.scalar_tensor_tensor(
                rv7[:, :, 0:g], pp[:], -1.0, u[:],
                op0=mybir.AluOpType.add, op1=mybir.AluOpType.mult,
            )

            for k in range(KCH):
                nc.tensor.matmul(
                    cls_ps[0:7, 0 : C * g],
                    rv7[:, k, :],
                    tt_sb[:, k, 0:p],
                    start=(mm_idx == 0),
                    stop=(mm_idx == n_cls_mm - 1),
                    skip_group_check=True,
                )
                mm_idx += 1

        cls_sb = const.tile([7, G_FULL * C], f32)
        nc.vector.tensor_copy(cls_sb[:], cls_ps[:])
        nc.sync.dma_start(out_d[:], cls_sb[:])

    return nc


def make_consts():
    s1 = np.zeros((G_FULL * C, G_FULL), dtype=ml_dtypes.bfloat16)
    for g in range(G_FULL):
        s1[g * C : (g + 1) * C, g] = 1
    ident = np.eye(128, dtype=ml_dtypes.bfloat16)
    return s1, ident


def finalize(cls_sum, hw_px=HW, n_cores=B):
    """Host-side fold of the per-core partials into the loss."""
    if cls_sum.shape[0] == 128:
        cls_sum = sum(cls_sum[32 * j : 32 * j + 7] for j in range(4))
    class_sum = np.zeros(C, dtype=np.float64)
    n_c = np.zeros(C, dtype=np.float64)
    for g in range(G_FULL):
        class_sum += cls_sum[g, g * C : (g + 1) * C]
        n_c += cls_sum[6, g * C : (g + 1) * C]
    n = float(n_cores) * float(hw_px)
    beta = np.float64(np.float32((n - 1.0) / n))
    w = (1.0 - beta) / (1.0 - np.power(beta, n_c) + 1e-6)
    return np.float32(-(w * class_sum).sum() / n)


_NC_CACHE = {}


def _get_nc():
    if "nc" not in _NC_CACHE:
        nc = build_nc()
        nc.finalize()
        _NC_CACHE["nc"] = nc
    return _NC_CACHE["nc"]


LAST_PROFILE = {}


def _install_trace_shims():
    """Provide the NTFF profile hook this image's antenv lacks, and keep
    artifact upload local."""
    import sys as _s
    import types

    try:
        import antenv.axon_hooks  # noqa: F401
    except ImportError:
        if "/root/.axon_site" not in _s.path:
            _s.path.insert(0, "/root/.axon_site")
        from trn_agent_boot.trn_boot import _ntff_profile_via_ctypes

        hook = _ntff_profile_via_ctypes("/opt/axon/libaxon_pjrt.so")
        mod = types.ModuleType("antenv.axon_hooks")
        mod.get_axon_ntff_profile_hook = lambda: hook
        mod.set_axon_ntff_profile_hook = lambda h: None
        _s.modules["antenv.axon_hooks"] = mod
    import concourse.bass_utils as bu

    bu.upload_artifacts = lambda tmpdir: str(tmpdir)


def kernel(pred, target, _trace=False):
    pred = np.ascontiguousarray(np.asarray(pred, dtype=np.float32)).reshape(B, C, HW)
    target = np.ascontiguousarray(np.asarray(target, dtype=np.float32)).reshape(
        B, C, HW
    )
    from concourse.bass_utils import run_bass_kernel_spmd

    nc = _get_nc()
    s1, ident = make_consts()
    in_maps = [
        {"pred": pred[i], "target": target[i], "s1": s1, "ident": ident}
        for i in range(B)
    ]
    kw = {}
    if _trace:
        import os

        import shutil

        _install_trace_shims()
        kw = {"tmpdir": "/root/problem/trace_out"}
        shutil.rmtree(kw["tmpdir"], ignore_errors=True)
        os.makedirs(kw["tmpdir"], exist_ok=True)
    rr = run_bass_kernel_spmd(nc, in_maps, list(range(B)), trace=_trace, **kw)
    LAST_PROFILE["exec_time_ns"] = rr.exec_time_ns
    LAST_PROFILE["trace"] = rr.instructions_and_trace
    LAST_PROFILE["profile_json"] = rr.profile_json
    cls = np.zeros((7, G_FULL * C), dtype=np.float64)
    for r in rr.results:
        cls += r["out"].astype(np.float64)
    return finalize(cls)
